# revision 26
# baseline (speedup 1.0000x reference)
"""Cross-attention kernel for Trainium2, 8 NeuronCores, data-parallel over batch.

Per-core computation (one batch b):
  image_norm = LN(image_features[b]); text_norm = LN(text_features[b])
  ip = image_norm @ W_img^T + b_img ; tp = text_norm @ W_txt^T + b_txt
  attn = softmax(ip @ tp^T / sqrt(D))
  image_out = attn @ tp ; text_out = attn^T @ ip

Key idea: LN is folded into the projection EVACUATION via a rank-1 update,
so the matmul-critical path is just load -> fp16 cast -> xbar transpose:
  proj[e,s] = rstd_s * G[e,s] - (rstd_s*mu_s) * w1[e] + b'[e]
  G = raw_x16 @ (W*ln_w)^T,  w1[e] = sum_d ln_w[d]*W[e,d],
  b' = ln_b @ W^T + b
Per-row LN stats are computed off the critical path, broadcast along
partitions via a tiny DRAM gather/replicate, and applied to PSUM in place.

All matmuls fp16 (fp32 PSUM); softmax fp32; softmax max-subtraction skipped
(logits ~N(0,0.33)). All layout changes are SBUF->SBUF DMA xbar transposes.
A (16 s-tiles x 2048) is fully SBUF-resident so text_out is a single
accumulation pass with no DRAM scratch. Outputs evacuate PSUM->fp16 and
cast-store fp16->fp32 via SWDGE.
"""

import os
import sys

import numpy as np

for _p in ("/opt/trn_rl_repo", "/root/.axon_site/_ro/trn_rl_repo"):
    if os.path.isdir(_p) and _p not in sys.path:
        sys.path.insert(0, _p)

import concourse.bass as bass  # noqa: E402
import concourse.mybir as mybir  # noqa: E402
import concourse.tile as tile  # noqa: E402
from concourse import bacc  # noqa: E402
from concourse.bass_utils import run_bass_kernel_spmd  # noqa: E402

F32 = mybir.dt.float32
DT = mybir.dt.float16

P = 128
S = 2048
D = 1024
ST = S // P   # 16 s-tiles per side
KT = D // P   # 8 contraction sub-tiles / e-tiles
CH = 512      # matmul moving free-dim chunk
NCH = S // CH # 4 chunks over s/t
DCH = D // CH # 2 chunks over d
TPC = ST // NCH  # 4 s-tiles per chunk
EPS = 1e-5
SCALE = float(D) ** -0.5
NCORES = 8

ACTF = mybir.ActivationFunctionType
ALU = mybir.AluOpType
AXL = mybir.AxisListType


def _body(tc):
    nc = tc.nc
    x_img = nc.dram_tensor("image_features", [S, D], F32, kind="ExternalInput").ap()
    x_txt = nc.dram_tensor("text_features", [S, D], F32, kind="ExternalInput").ap()
    lnw = nc.dram_tensor("ln_w", [D], F32, kind="ExternalInput").ap()
    lnb = nc.dram_tensor("ln_b", [D], F32, kind="ExternalInput").ap()
    W_img = nc.dram_tensor("W_img", [D, D], F32, kind="ExternalInput").ap()
    b_img = nc.dram_tensor("b_img", [D], F32, kind="ExternalInput").ap()
    W_txt = nc.dram_tensor("W_txt", [D, D], F32, kind="ExternalInput").ap()
    b_txt = nc.dram_tensor("b_txt", [D], F32, kind="ExternalInput").ap()
    io_out = nc.dram_tensor("image_out", [S, D], F32, kind="ExternalOutput").ap()
    to_out = nc.dram_tensor("text_out", [S, D], F32, kind="ExternalOutput").ap()

    # long-lived pools on the left SBUF stack; transient pools on the right
    # stack (LIFO per side) so they release while the left survives
    persist = tc.alloc_tile_pool(name="persist", bufs=1)
    stats = tc.alloc_tile_pool(name="stats", bufs=4)
    pT = tc.alloc_tile_pool(name="pT", bufs=1, side="right")
    wT = tc.alloc_tile_pool(name="wT", bufs=1, side="right")
    xT = tc.alloc_tile_pool(name="xT", bufs=4, side="right")
    bcast = tc.alloc_tile_pool(name="bcast", bufs=6, side="right")
    xr = tc.alloc_tile_pool(name="xr", bufs=6, side="right")
    x16p = tc.alloc_tile_pool(name="x16", bufs=5, side="right")
    wraw = tc.alloc_tile_pool(name="wraw", bufs=1, side="right")
    psA = tc.alloc_tile_pool(name="psA", bufs=4, space="PSUM")
    psB = tc.alloc_tile_pool(name="psB", bufs=4, space="PSUM")
    dram = tc.alloc_tile_pool(name="dram", bufs=1, space="DRAM")
    # stat-gather scratch: row per (side, stat, chunk) = side*8 + stat*4 + c
    dscr = dram.tile([16, TPC, P], F32, name="dscr")

    eps_t = persist.tile([P, 1], F32, tag="eps")
    nc.vector.memset(eps_t[:], EPS)
    scale_t = persist.tile([P, 1], F32, tag="scl")
    nc.vector.memset(scale_t[:], SCALE)
    # bw strip: bw_t[p, k, 0] = ln_b[k*128+p], bw_t[p, k, 1] = ln_w[k*128+p]
    bw_t = persist.tile([P, KT, 2], DT, tag="bw")
    nc.gpsimd.dma_start(bw_t[:, :, 0:1], lnb.rearrange("(k p) -> p k", p=P))
    nc.gpsimd.dma_start(bw_t[:, :, 1:2], lnw.rearrange("(k p) -> p k", p=P))
    bprime = [persist.tile([P, KT], F32, tag=f"bp{i}", name=f"bprime{i}") for i in range(2)]
    w1n = [persist.tile([P, KT], F32, tag=f"w1n{i}", name=f"w1n{i}") for i in range(2)]
    bpart = [persist.tile([P, KT], F32, tag=f"bpt{i}", name=f"bpart{i}") for i in range(2)]
    nc.sync.dma_start(bpart[1][:], b_txt.rearrange("(k p) -> p k", p=P))
    nc.sync.dma_start(bpart[0][:], b_img.rearrange("(k p) -> p k", p=P))
    rinv = persist.tile([P, ST], F32, tag="rinv")
    # per-side LN stat tiles: mvc[:, 0, i] = mean of s-tile i, [:, 1, i] = var
    mvc = [persist.tile([P, 2, ST], F32, tag=f"mvc{s}", name=f"mvc{s}") for s in range(2)]
    rstdc = [persist.tile([P, ST], F32, tag=f"rstd{s}", name=f"rstdc{s}") for s in range(2)]
    rmuc = [persist.tile([P, ST], F32, tag=f"rmu{s}", name=f"rmuc{s}") for s in range(2)]

    tpT = pT.tile([P, KT, S], DT, tag="tpT")

    # ---------------- helper emitters (each touches one engine queue) -------
    def em_w_load(wi, W_d):
        w16 = wraw.tile([P, KT, D], DT, tag="w16", name=f"w16_{wi}")
        nc.gpsimd.dma_start(w16[:, :, :], W_d.rearrange("(et p) d -> p et d", p=P))
        return w16

    def em_lnwbc():
        lnw_bc = xr.tile([P, D], DT, tag="lnwbc", name="lnw_bc")
        src = bass.AP(tensor=lnw.tensor, offset=lnw.offset, ap=[[0, P]] + list(lnw.ap))
        nc.gpsimd.dma_start(lnw_bc[:], src)
        return lnw_bc

    def em_w_transpose_half(wi, w16, WT4, h):
        H = KT // 2
        nc.sync.dma_start_transpose(
            WT4[:, h * H : (h + 1) * H, :, :], w16[:, h * H : (h + 1) * H, :]
        )

    xr_tiles = {}
    x16_tiles = {}

    def em_x_load(side, x_d, i):
        t = xr.tile([P, D], F32, tag="xr", name=f"xr_{side}_{i}")
        nc.scalar.dma_start(t[:], x_d[i * P : (i + 1) * P, :])
        xr_tiles[(side, i)] = t

    def em_x_cast(side, i):
        t = x16p.tile([P, D], DT, tag="x16", name=f"x16_{side}_{i}")
        nc.vector.scalar_tensor_tensor(
            t[:], xr_tiles[(side, i)][:], 1.0, lnw_bc[:],
            op0=ALU.mult, op1=ALU.mult,
        )
        x16_tiles[(side, i)] = t

    def em_x_bn(side, i):
        st = stats.tile([P, 2, 6], F32, tag="bnst")
        t = xr_tiles[(side, i)]
        nc.vector.bn_stats(out=st[:, 0, :], in_=t[:, 0:512])
        nc.vector.bn_stats(out=st[:, 1, :], in_=t[:, 512:1024])
        nc.vector.bn_aggr(out=mvc[side][:, :, i], in_=st[:])

    def em_rsq(side, c):
        nc.scalar.activation(
            rstdc[side][:, c * TPC : (c + 1) * TPC],
            mvc[side][:, 1, c * TPC : (c + 1) * TPC],
            ACTF.Sqrt, bias=eps_t[:], scale=1.0,
        )

    def em_rmu(side, c):
        nc.vector.reciprocal(
            rstdc[side][:, c * TPC : (c + 1) * TPC],
            rstdc[side][:, c * TPC : (c + 1) * TPC],
        )
        nc.vector.scalar_tensor_tensor(
            rmuc[side][:, c * TPC : (c + 1) * TPC],
            mvc[side][:, 0, c * TPC : (c + 1) * TPC],
            1.0,
            rstdc[side][:, c * TPC : (c + 1) * TPC],
            op0=ALU.mult, op1=ALU.mult,
        )

    def em_gath(side, c):
        # stat columns [128, TPC] -> DRAM row [TPC*128] with elem (p,t)->t*128+p
        for stat, col in ((0, rstdc[side]), (1, rmuc[side])):
            r = side * 8 + stat * TPC + c
            base = dscr[r : r + 1, :, :]
            dst = bass.AP(tensor=base.tensor, offset=base.offset, ap=[[1, P], [P, TPC]])
            nc.sync.dma_start(dst, col[:, c * TPC : (c + 1) * TPC])

    bc_tiles = {}

    def em_bcast(side, c):
        for stat in range(2):
            r = side * 8 + stat * TPC + c
            base = dscr[r : r + 1, :, :]
            src = bass.AP(tensor=base.tensor, offset=base.offset, ap=[[0, P], [1, CH]])
            t = bcast.tile([P, CH], F32, tag="bc", name=f"bc_{side}_{stat}_{c}")
            nc.gpsimd.dma_start(t[:], src)
            bc_tiles[(side, stat, c)] = t

    xT_cs = {}

    def em_x_T(side, i):
        c, st_loc = divmod(i, TPC)
        if (side, c) not in xT_cs:
            xT_cs[(side, c)] = xT.tile([P, KT, CH], DT, tag="xTc", name=f"xT_{side}_{c}")
        nc.sync.dma_start_transpose(
            xT_cs[(side, c)][:, :, st_loc * P : (st_loc + 1) * P],
            x16_tiles[(side, i)][:],
        )

    def em_proj_cp(wi, side, WT4, cp, out_pT, extras):
        for et in range(KT):
            pp0 = psA.tile([P, CH], F32, tag="mm", name=f"pp_{side}_{cp}_{et}_0")
            pp1 = psB.tile([P, CH], F32, tag="io", name=f"pp_{side}_{cp}_{et}_1")
            pps = [pp0, pp1]
            vwps = None
            if cp == 0:
                vwps = psA.tile([P, CH], F32, tag="mm", name=f"vw_{side}_{et}")
            for kk in range(KT):
                for cc in range(2):
                    nc.tensor.matmul(
                        pps[cc][:],
                        lhsT=WT4[:, et, kk, :],
                        rhs=xT_cs[(side, cp * 2 + cc)][:, kk, :],
                        start=(kk == 0),
                        stop=(kk == KT - 1),
                    )
                if cp == 0:
                    # same stationary weights: piggyback ln_b@W^T and
                    # sum_d ln_w*W row-reductions as an N=2 matmul
                    nc.tensor.matmul(
                        vwps[:, 0:2],
                        lhsT=WT4[:, et, kk, :],
                        rhs=bw_t[:, kk, :],
                        start=(kk == 0),
                        stop=(kk == KT - 1),
                    )
            if cp == 0:
                nc.vector.scalar_tensor_tensor(
                    bprime[wi][:, et : et + 1], vwps[:, 0:1], 1.0,
                    bpart[wi][:, et : et + 1], op0=ALU.mult, op1=ALU.add,
                )
                nc.vector.tensor_scalar_mul(
                    w1n[wi][:, et : et + 1], vwps[:, 1:2], -1.0
                )
            for cc in range(2):
                c = cp * 2 + cc
                # psum = psum*rstd ; psum += rmu*(-w1) ; out = psum + b' (fp16)
                nc.vector.scalar_tensor_tensor(
                    pps[cc][:], pps[cc][:], 1.0, bc_tiles[(side, 0, c)][:],
                    op0=ALU.mult, op1=ALU.mult,
                )
                nc.vector.scalar_tensor_tensor(
                    pps[cc][:], bc_tiles[(side, 1, c)][:],
                    w1n[wi][:, et : et + 1], pps[cc][:],
                    op0=ALU.mult, op1=ALU.add,
                )
                nc.scalar.activation(
                    out_pT[:, et, c * CH : (c + 1) * CH], pps[cc][:],
                    ACTF.Identity, bias=bprime[wi][:, et : et + 1], scale=1.0,
                )
            for fn in extras.get(et, ()):
                fn()

    # ---------------- emission schedule ------------------------------------
    w16_t = em_w_load(1, W_txt)
    lnw_bc = em_lnwbc()
    for i in range(ST):
        em_x_load(0, x_txt, i)
    WT4_t = wT.tile([P, KT, KT, P], DT, tag="WT1", name="WT4_txt")
    em_w_transpose_half(1, w16_t, WT4_t, 0)
    em_w_transpose_half(1, w16_t, WT4_t, 1)
    for i in range(4):
        em_x_cast(0, i)
        em_x_bn(0, i)
    em_rsq(0, 0)
    em_rmu(0, 0)
    for i in range(4, 8):
        em_x_cast(0, i)
        em_x_bn(0, i)
    em_rsq(0, 1)
    em_rmu(0, 1)
    for i in range(8, ST):
        em_x_cast(0, i)
    for i in range(0, 4):
        em_x_T(0, i)
    em_gath(0, 0)
    for i in range(4, 8):
        em_x_T(0, i)
    em_gath(0, 1)
    for i in range(8, ST):
        em_x_T(0, i)
    for i in range(ST):
        em_x_load(1, x_img, i)
    em_bcast(0, 0)
    em_bcast(0, 1)
    w16_i = em_w_load(0, W_img)

    extras_cp0 = {
        0: (lambda: em_x_bn(0, 8), lambda: em_x_bn(0, 9)),
        1: (lambda: em_x_bn(0, 10), lambda: em_x_bn(0, 11)),
        2: (lambda: em_x_bn(0, 12), lambda: em_x_bn(0, 13), lambda: em_rsq(0, 2)),
        3: (lambda: em_x_bn(0, 14), lambda: em_x_bn(0, 15),
            lambda: em_rmu(0, 2), lambda: em_gath(0, 2)),
        4: (lambda: em_bcast(0, 2), lambda: em_rsq(0, 3)),
        5: (lambda: em_rmu(0, 3), lambda: em_gath(0, 3), lambda: em_bcast(0, 3)),
    }
    em_proj_cp(1, 0, WT4_t, 0, tpT, extras_cp0)

    WT4_i = wT.tile([P, KT, KT, P], DT, tag="WT0", name="WT4_img")
    extras_cp1 = {
        0: (lambda: em_w_transpose_half(0, w16_i, WT4_i, 0),
            lambda: em_x_cast(1, 0), lambda: em_x_bn(1, 0), lambda: em_x_T(1, 0)),
        1: (lambda: em_w_transpose_half(0, w16_i, WT4_i, 1),
            lambda: em_x_cast(1, 1), lambda: em_x_bn(1, 1), lambda: em_x_T(1, 1)),
        2: (lambda: em_x_cast(1, 2), lambda: em_x_bn(1, 2), lambda: em_x_T(1, 2)),
        3: (lambda: em_x_cast(1, 3), lambda: em_x_bn(1, 3), lambda: em_x_T(1, 3),
            lambda: em_rsq(1, 0)),
        4: (lambda: em_x_cast(1, 4), lambda: em_x_bn(1, 4), lambda: em_x_T(1, 4),
            lambda: em_rmu(1, 0), lambda: em_gath(1, 0)),
        5: (lambda: em_x_cast(1, 5), lambda: em_x_bn(1, 5), lambda: em_x_T(1, 5),
            lambda: em_bcast(1, 0)),
        6: (lambda: em_x_cast(1, 6), lambda: em_x_bn(1, 6), lambda: em_x_T(1, 6),
            lambda: em_rsq(1, 1)),
        7: (lambda: em_x_cast(1, 7), lambda: em_x_bn(1, 7), lambda: em_x_T(1, 7),
            lambda: em_rmu(1, 1), lambda: em_gath(1, 1)),
    }
    em_proj_cp(1, 0, WT4_t, 1, tpT, extras_cp1)
    em_bcast(1, 1)
    wraw.release()

    ipT = pT.tile([P, KT, S], DT, tag="ipT")
    extras_icp0 = {
        0: (lambda: em_x_cast(1, 8), lambda: em_x_bn(1, 8), lambda: em_x_T(1, 8)),
        1: (lambda: em_x_cast(1, 9), lambda: em_x_bn(1, 9), lambda: em_x_T(1, 9)),
        2: (lambda: em_x_cast(1, 10), lambda: em_x_bn(1, 10), lambda: em_x_T(1, 10)),
        3: (lambda: em_x_cast(1, 11), lambda: em_x_bn(1, 11), lambda: em_x_T(1, 11),
            lambda: em_rsq(1, 2)),
        4: (lambda: em_x_cast(1, 12), lambda: em_x_bn(1, 12), lambda: em_x_T(1, 12),
            lambda: em_rmu(1, 2), lambda: em_gath(1, 2), lambda: em_bcast(1, 2)),
        5: (lambda: em_x_cast(1, 13), lambda: em_x_bn(1, 13), lambda: em_x_T(1, 13)),
        6: (lambda: em_x_cast(1, 14), lambda: em_x_bn(1, 14), lambda: em_x_T(1, 14),
            lambda: em_rsq(1, 3)),
        7: (lambda: em_x_cast(1, 15), lambda: em_x_bn(1, 15), lambda: em_x_T(1, 15),
            lambda: em_rmu(1, 3), lambda: em_gath(1, 3), lambda: em_bcast(1, 3)),
    }
    em_proj_cp(0, 1, WT4_i, 0, ipT, extras_icp0)
    em_proj_cp(0, 1, WT4_i, 1, ipT, {})

    x16p.release()
    xr.release()
    bcast.release()
    xT.release()
    wT.release()

    tp_pool = tc.alloc_tile_pool(name="tp", bufs=1)
    tp = tp_pool.tile([P, ST, D], DT, tag="tp")
    for et in range(KT):
        nc.sync.dma_start_transpose(tp[:, :, et * P : (et + 1) * P], tpT[:, et, :])

    attn_pool = tc.alloc_tile_pool(name="attn", bufs=1)
    A = attn_pool.tile([P, ST, S], DT, tag="A")
    ip = attn_pool.tile([P, ST, D], DT, tag="ip")
    evq = tc.alloc_tile_pool(name="evq", bufs=2)
    outs = tc.alloc_tile_pool(name="outs", bufs=2)

    # ---- attention: QK + exp + A^T, image_out pipelined one s-tile behind --
    def _emit_io(m, at_full):
        iops = [
            psB.tile([P, CH], F32, tag="io", name=f"io_{m}_{dci}")
            for dci in range(DCH)
        ]
        for tt in range(ST):
            for dc in range(DCH):
                nc.tensor.matmul(
                    iops[dc][:],
                    lhsT=at_full[:, tt, :],
                    rhs=tp[:, tt, dc * CH : (dc + 1) * CH],
                    start=(tt == 0),
                    stop=(tt == ST - 1),
                )
        iosb = outs.tile([P, D], DT, tag="osb", name=f"iosb_{m}")
        for dc in range(DCH):
            nc.vector.tensor_copy(iosb[:, dc * CH : (dc + 1) * CH], iops[dc][:])
        nc.gpsimd.dma_start(io_out[m * P : (m + 1) * P, :], iosb[:])

    pending_io = None
    for m in range(ST):
        qps = [
            psA.tile([P, CH], F32, tag="mm", name=f"qk_{m}_{ci}")
            for ci in range(NCH)
        ]
        for kk in range(KT):
            for ci in range(NCH):
                nc.tensor.matmul(
                    qps[ci][:],
                    lhsT=ipT[:, kk, m * P : (m + 1) * P],
                    rhs=tpT[:, kk, ci * CH : (ci + 1) * CH],
                    start=(kk == 0),
                    stop=(kk == KT - 1),
                )
        rs4 = stats.tile([P, NCH], F32, tag="rs4")
        for ci in range(NCH):
            nc.scalar.activation(
                A[:, m, ci * CH : (ci + 1) * CH],
                qps[ci][:],
                ACTF.Exp,
                bias=0.0,
                scale=scale_t[:],
                accum_out=rs4[:, ci : ci + 1],
            )
        rsum = stats.tile([P, 1], F32, tag="rsum")
        nc.vector.reduce_sum(rsum[:], rs4[:], axis=AXL.X)
        nc.vector.reciprocal(rinv[:, m : m + 1], rsum[:])
        nc.vector.tensor_scalar_mul(A[:, m, :], A[:, m, :], rinv[:, m : m + 1])
        at_full = evq.tile([P, ST, P], DT, tag="at", name=f"at_{m}")
        nc.sync.dma_start_transpose(at_full[:, :, :], A[:, m, :])
        # image natural layout [s, e] built during the QK phase (sync queue)
        if m < KT:
            nc.sync.dma_start_transpose(ip[:, :, m * P : (m + 1) * P], ipT[:, m, :])
        if pending_io is not None:
            _emit_io(*pending_io)
        pending_io = (m, at_full)
    _emit_io(*pending_io)
    pending_io = None

    # ---- text_out: single pass, full-A ----
    for tt in range(ST):
        tops = [
            psB.tile([P, CH], F32, tag="io", name=f"to_{tt}_{dci}")
            for dci in range(DCH)
        ]
        for ss in range(ST):
            for dc in range(DCH):
                nc.tensor.matmul(
                    tops[dc][:],
                    lhsT=A[:, ss, tt * P : (tt + 1) * P],
                    rhs=ip[:, ss, dc * CH : (dc + 1) * CH],
                    start=(ss == 0),
                    stop=(ss == ST - 1),
                )
        tosb = outs.tile([P, D], DT, tag="osb", name=f"tosb_{tt}")
        for dc in range(DCH):
            nc.vector.tensor_copy(tosb[:, dc * CH : (dc + 1) * CH], tops[dc][:])
        nc.gpsimd.dma_start(to_out[tt * P : (tt + 1) * P, :], tosb[:])

    for p in (outs, evq, attn_pool, tp_pool, pT, psB, psA, dram, stats, persist):
        p.release()


_NC_CACHE = {}


def build_nc():
    if "nc" not in _NC_CACHE:
        nc = bacc.Bacc("TRN2", target_bir_lowering=False, debug=False)
        with tile.TileContext(nc) as tc:
            _body(tc)
        nc.compile()
        _NC_CACHE["nc"] = nc
    return _NC_CACHE["nc"]


def _in_maps(image_features, text_features, ln_w, ln_b, W_img, b_img, W_txt, b_txt):
    f32 = lambda a: np.ascontiguousarray(np.asarray(a), dtype=np.float32)
    shared = {
        "ln_w": f32(ln_w),
        "ln_b": f32(ln_b),
        "W_img": f32(W_img),
        "b_img": f32(b_img),
        "W_txt": f32(W_txt),
        "b_txt": f32(b_txt),
    }
    maps = []
    for b in range(NCORES):
        m = dict(shared)
        m["image_features"] = f32(image_features[b])
        m["text_features"] = f32(text_features[b])
        maps.append(m)
    return maps


def run(inputs, trace=False, tmpdir=None):
    nc = build_nc()
    maps = _in_maps(**inputs)
    res = run_bass_kernel_spmd(
        nc, maps, core_ids=list(range(NCORES)), trace=trace, tmpdir=tmpdir
    )
    io = np.stack([res.results[b]["image_out"] for b in range(NCORES)])
    to = np.stack([res.results[b]["text_out"] for b in range(NCORES)])
    return (io, to), res


def kernel(**inputs):
    out, _ = run(inputs, trace=False)
    return out


# revision 27
# speedup vs baseline: 1.0469x; 1.0469x over previous
"""Cross-attention kernel for Trainium2, 8 NeuronCores, data-parallel over batch.

Per-core computation (one batch b):
  image_norm = LN(image_features[b]); text_norm = LN(text_features[b])
  ip = image_norm @ W_img^T + b_img ; tp = text_norm @ W_txt^T + b_txt
  attn = softmax(ip @ tp^T / sqrt(D))
  image_out = attn @ tp ; text_out = attn^T @ ip

Key idea: LN is folded into the projection EVACUATION via a rank-1 update,
so the matmul-critical path is just load -> fp16 cast -> xbar transpose:
  proj[e,s] = rstd_s * G[e,s] - (rstd_s*mu_s) * w1[e] + b'[e]
  G = raw_x16 @ (W*ln_w)^T,  w1[e] = sum_d ln_w[d]*W[e,d],
  b' = ln_b @ W^T + b
Per-row LN stats are computed off the critical path, broadcast along
partitions via a tiny DRAM gather/replicate, and applied to PSUM in place.

All matmuls fp16 (fp32 PSUM); softmax fp32; softmax max-subtraction skipped
(logits ~N(0,0.33)). All layout changes are SBUF->SBUF DMA xbar transposes.
A (16 s-tiles x 2048) is fully SBUF-resident so text_out is a single
accumulation pass with no DRAM scratch. Outputs evacuate PSUM->fp16 and
cast-store fp16->fp32 via SWDGE.
"""

import os
import sys

import numpy as np

for _p in ("/opt/trn_rl_repo", "/root/.axon_site/_ro/trn_rl_repo"):
    if os.path.isdir(_p) and _p not in sys.path:
        sys.path.insert(0, _p)

import concourse.bass as bass  # noqa: E402
import concourse.mybir as mybir  # noqa: E402
import concourse.tile as tile  # noqa: E402
from concourse import bacc  # noqa: E402
from concourse.bass_utils import run_bass_kernel_spmd  # noqa: E402

F32 = mybir.dt.float32
DT = mybir.dt.float16

P = 128
S = 2048
D = 1024
ST = S // P   # 16 s-tiles per side
KT = D // P   # 8 contraction sub-tiles / e-tiles
CH = 512      # matmul moving free-dim chunk
NCH = S // CH # 4 chunks over s/t
DCH = D // CH # 2 chunks over d
TPC = ST // NCH  # 4 s-tiles per chunk
EPS = 1e-5
SCALE = float(D) ** -0.5
NCORES = 8

ACTF = mybir.ActivationFunctionType
ALU = mybir.AluOpType
AXL = mybir.AxisListType


def _body(tc):
    nc = tc.nc
    x_img = nc.dram_tensor("image_features", [S, D], F32, kind="ExternalInput").ap()
    x_txt = nc.dram_tensor("text_features", [S, D], F32, kind="ExternalInput").ap()
    lnw = nc.dram_tensor("ln_w", [D], F32, kind="ExternalInput").ap()
    lnb = nc.dram_tensor("ln_b", [D], F32, kind="ExternalInput").ap()
    W_img = nc.dram_tensor("W_img", [D, D], F32, kind="ExternalInput").ap()
    b_img = nc.dram_tensor("b_img", [D], F32, kind="ExternalInput").ap()
    W_txt = nc.dram_tensor("W_txt", [D, D], F32, kind="ExternalInput").ap()
    b_txt = nc.dram_tensor("b_txt", [D], F32, kind="ExternalInput").ap()
    io_out = nc.dram_tensor("image_out", [S, D], F32, kind="ExternalOutput").ap()
    to_out = nc.dram_tensor("text_out", [S, D], F32, kind="ExternalOutput").ap()

    # long-lived pools on the left SBUF stack; transient pools on the right
    # stack (LIFO per side) so they release while the left survives
    persist = tc.alloc_tile_pool(name="persist", bufs=1)
    stats = tc.alloc_tile_pool(name="stats", bufs=4)
    pT = tc.alloc_tile_pool(name="pT", bufs=1, side="right")
    wT = tc.alloc_tile_pool(name="wT", bufs=1, side="right")
    xT = tc.alloc_tile_pool(name="xT", bufs=4, side="right")
    bcast = tc.alloc_tile_pool(name="bcast", bufs=6, side="right")
    xr = tc.alloc_tile_pool(name="xr", bufs=6, side="right")
    x16p = tc.alloc_tile_pool(name="x16", bufs=5, side="right")
    wraw = tc.alloc_tile_pool(name="wraw", bufs=1, side="right")
    psA = tc.alloc_tile_pool(name="psA", bufs=4, space="PSUM")
    psB = tc.alloc_tile_pool(name="psB", bufs=4, space="PSUM")
    dram = tc.alloc_tile_pool(name="dram", bufs=1, space="DRAM")
    # stat-gather scratch: row per (side, stat, chunk) = side*8 + stat*4 + c
    dscr = dram.tile([16, TPC, P], F32, name="dscr")

    eps_t = persist.tile([P, 1], F32, tag="eps")
    nc.vector.memset(eps_t[:], EPS)
    scale_t = persist.tile([P, 1], F32, tag="scl")
    nc.vector.memset(scale_t[:], SCALE)
    # bw strip: bw_t[p, k, 0] = ln_b[k*128+p], bw_t[p, k, 1] = ln_w[k*128+p]
    bw_t = persist.tile([P, KT, 2], DT, tag="bw")
    nc.gpsimd.dma_start(bw_t[:, :, 0:1], lnb.rearrange("(k p) -> p k", p=P))
    nc.gpsimd.dma_start(bw_t[:, :, 1:2], lnw.rearrange("(k p) -> p k", p=P))
    bprime = [persist.tile([P, KT], F32, tag=f"bp{i}", name=f"bprime{i}") for i in range(2)]
    w1n = [persist.tile([P, KT], F32, tag=f"w1n{i}", name=f"w1n{i}") for i in range(2)]
    bpart = [persist.tile([P, KT], F32, tag=f"bpt{i}", name=f"bpart{i}") for i in range(2)]
    nc.sync.dma_start(bpart[1][:], b_txt.rearrange("(k p) -> p k", p=P))
    nc.sync.dma_start(bpart[0][:], b_img.rearrange("(k p) -> p k", p=P))
    rinv = persist.tile([P, ST], F32, tag="rinv")
    # per-side LN stat tiles: mvc[:, 0, i] = mean of s-tile i, [:, 1, i] = var
    mvc = [persist.tile([P, ST, 2], F32, tag=f"mvc{s}", name=f"mvc{s}") for s in range(2)]
    rstdc = [persist.tile([P, ST], F32, tag=f"rstd{s}", name=f"rstdc{s}") for s in range(2)]
    rmuc = [persist.tile([P, ST], F32, tag=f"rmu{s}", name=f"rmuc{s}") for s in range(2)]

    tpT = pT.tile([P, KT, S], DT, tag="tpT")

    # ---------------- helper emitters (each touches one engine queue) -------
    def em_w_load(wi, W_d):
        w16 = wraw.tile([P, KT, D], DT, tag="w16", name=f"w16_{wi}")
        nc.gpsimd.dma_start(w16[:, :, :], W_d.rearrange("(et p) d -> p et d", p=P))
        return w16

    def em_lnwbc():
        lnw_bc = xr.tile([P, D], DT, tag="lnwbc", name="lnw_bc")
        src = bass.AP(tensor=lnw.tensor, offset=lnw.offset, ap=[[0, P]] + list(lnw.ap))
        nc.gpsimd.dma_start(lnw_bc[:], src)
        return lnw_bc

    def em_w_transpose_half(wi, w16, WT4, h):
        H = KT // 2
        nc.sync.dma_start_transpose(
            WT4[:, h * H : (h + 1) * H, :, :], w16[:, h * H : (h + 1) * H, :]
        )

    xr_tiles = {}
    x16_tiles = {}

    def em_x_load(side, x_d, i):
        t = xr.tile([P, D], F32, tag="xr", name=f"xr_{side}_{i}")
        nc.scalar.dma_start(t[:], x_d[i * P : (i + 1) * P, :])
        xr_tiles[(side, i)] = t

    def em_x_cast(side, i):
        t = x16p.tile([P, D], DT, tag="x16", name=f"x16_{side}_{i}")
        nc.vector.scalar_tensor_tensor(
            t[:], xr_tiles[(side, i)][:], 1.0, lnw_bc[:],
            op0=ALU.mult, op1=ALU.mult,
        )
        x16_tiles[(side, i)] = t

    def em_x_bn(side, i):
        st = stats.tile([P, 2, 6], F32, tag="bnst")
        t = xr_tiles[(side, i)]
        nc.vector.bn_stats(out=st[:, 0, :], in_=t[:, 0:512])
        nc.vector.bn_stats(out=st[:, 1, :], in_=t[:, 512:1024])
        nc.vector.bn_aggr(out=mvc[side][:, i, :], in_=st[:])

    def em_rsq(side, c):
        nc.scalar.activation(
            rstdc[side][:, c * TPC : (c + 1) * TPC],
            mvc[side][:, c * TPC : (c + 1) * TPC, 1:2],
            ACTF.Sqrt, bias=eps_t[:], scale=1.0,
        )

    def em_rmu(side, c):
        nc.vector.reciprocal(
            rstdc[side][:, c * TPC : (c + 1) * TPC],
            rstdc[side][:, c * TPC : (c + 1) * TPC],
        )
        nc.vector.scalar_tensor_tensor(
            rmuc[side][:, c * TPC : (c + 1) * TPC],
            mvc[side][:, c * TPC : (c + 1) * TPC, 0:1],
            1.0,
            rstdc[side][:, c * TPC : (c + 1) * TPC],
            op0=ALU.mult, op1=ALU.mult,
        )

    def em_gath(side, c):
        # stat columns [128, TPC] -> DRAM row [TPC*128] with elem (p,t)->t*128+p
        for stat, col in ((0, rstdc[side]), (1, rmuc[side])):
            r = side * 8 + stat * TPC + c
            base = dscr[r : r + 1, :, :]
            dst = bass.AP(tensor=base.tensor, offset=base.offset, ap=[[1, P], [P, TPC]])
            nc.sync.dma_start(dst, col[:, c * TPC : (c + 1) * TPC])

    bc_tiles = {}

    def em_bcast(side, c):
        for stat in range(2):
            r = side * 8 + stat * TPC + c
            base = dscr[r : r + 1, :, :]
            src = bass.AP(tensor=base.tensor, offset=base.offset, ap=[[0, P], [1, CH]])
            t = bcast.tile([P, CH], F32, tag="bc", name=f"bc_{side}_{stat}_{c}")
            nc.gpsimd.dma_start(t[:], src)
            bc_tiles[(side, stat, c)] = t

    xT_cs = {}

    def em_x_T(side, i):
        c, st_loc = divmod(i, TPC)
        if (side, c) not in xT_cs:
            xT_cs[(side, c)] = xT.tile([P, KT, CH], DT, tag="xTc", name=f"xT_{side}_{c}")
        nc.sync.dma_start_transpose(
            xT_cs[(side, c)][:, :, st_loc * P : (st_loc + 1) * P],
            x16_tiles[(side, i)][:],
        )

    def em_proj_cp(wi, side, WT4, cp, out_pT, extras):
        for et in range(KT):
            pp0 = psA.tile([P, CH], F32, tag="mm", name=f"pp_{side}_{cp}_{et}_0")
            pp1 = psB.tile([P, CH], F32, tag="io", name=f"pp_{side}_{cp}_{et}_1")
            pps = [pp0, pp1]
            vwps = None
            if cp == 0:
                vwps = psA.tile([P, CH], F32, tag="mm", name=f"vw_{side}_{et}")
            for kk in range(KT):
                for cc in range(2):
                    nc.tensor.matmul(
                        pps[cc][:],
                        lhsT=WT4[:, et, kk, :],
                        rhs=xT_cs[(side, cp * 2 + cc)][:, kk, :],
                        start=(kk == 0),
                        stop=(kk == KT - 1),
                    )
                if cp == 0:
                    # same stationary weights: piggyback ln_b@W^T and
                    # sum_d ln_w*W row-reductions as an N=2 matmul
                    nc.tensor.matmul(
                        vwps[:, 0:2],
                        lhsT=WT4[:, et, kk, :],
                        rhs=bw_t[:, kk, :],
                        start=(kk == 0),
                        stop=(kk == KT - 1),
                    )
            if cp == 0:
                nc.vector.scalar_tensor_tensor(
                    bprime[wi][:, et : et + 1], vwps[:, 0:1], 1.0,
                    bpart[wi][:, et : et + 1], op0=ALU.mult, op1=ALU.add,
                )
                nc.vector.tensor_scalar_mul(
                    w1n[wi][:, et : et + 1], vwps[:, 1:2], -1.0
                )
            for cc in range(2):
                c = cp * 2 + cc
                # psum = psum*rstd ; psum += rmu*(-w1) ; out = psum + b' (fp16)
                nc.vector.scalar_tensor_tensor(
                    pps[cc][:], pps[cc][:], 1.0, bc_tiles[(side, 0, c)][:],
                    op0=ALU.mult, op1=ALU.mult,
                )
                nc.vector.scalar_tensor_tensor(
                    pps[cc][:], bc_tiles[(side, 1, c)][:],
                    w1n[wi][:, et : et + 1], pps[cc][:],
                    op0=ALU.mult, op1=ALU.add,
                )
                nc.scalar.activation(
                    out_pT[:, et, c * CH : (c + 1) * CH], pps[cc][:],
                    ACTF.Identity, bias=bprime[wi][:, et : et + 1], scale=1.0,
                )
            for fn in extras.get(et, ()):
                fn()

    # ---------------- emission schedule ------------------------------------
    w16_t = em_w_load(1, W_txt)
    lnw_bc = em_lnwbc()
    for i in range(ST):
        em_x_load(0, x_txt, i)
    WT4_t = wT.tile([P, KT, KT, P], DT, tag="WT1", name="WT4_txt")
    em_w_transpose_half(1, w16_t, WT4_t, 0)
    em_w_transpose_half(1, w16_t, WT4_t, 1)
    for i in range(4):
        em_x_cast(0, i)
        em_x_bn(0, i)
    em_rsq(0, 0)
    em_rmu(0, 0)
    for i in range(4, 8):
        em_x_cast(0, i)
        em_x_bn(0, i)
    em_rsq(0, 1)
    em_rmu(0, 1)
    for i in range(8, ST):
        em_x_cast(0, i)
    for i in range(0, 4):
        em_x_T(0, i)
    em_gath(0, 0)
    for i in range(4, 8):
        em_x_T(0, i)
    em_gath(0, 1)
    for i in range(8, ST):
        em_x_T(0, i)
    for i in range(ST):
        em_x_load(1, x_img, i)
    em_bcast(0, 0)
    em_bcast(0, 1)
    w16_i = em_w_load(0, W_img)

    extras_cp0 = {
        0: (lambda: em_x_bn(0, 8), lambda: em_x_bn(0, 9)),
        1: (lambda: em_x_bn(0, 10), lambda: em_x_bn(0, 11)),
        2: (lambda: em_x_bn(0, 12), lambda: em_x_bn(0, 13), lambda: em_rsq(0, 2)),
        3: (lambda: em_x_bn(0, 14), lambda: em_x_bn(0, 15),
            lambda: em_rmu(0, 2), lambda: em_gath(0, 2)),
        4: (lambda: em_bcast(0, 2), lambda: em_rsq(0, 3)),
        5: (lambda: em_rmu(0, 3), lambda: em_gath(0, 3), lambda: em_bcast(0, 3)),
    }
    em_proj_cp(1, 0, WT4_t, 0, tpT, extras_cp0)

    WT4_i = wT.tile([P, KT, KT, P], DT, tag="WT0", name="WT4_img")
    extras_cp1 = {
        0: (lambda: em_w_transpose_half(0, w16_i, WT4_i, 0),
            lambda: em_x_cast(1, 0), lambda: em_x_bn(1, 0), lambda: em_x_T(1, 0)),
        1: (lambda: em_w_transpose_half(0, w16_i, WT4_i, 1),
            lambda: em_x_cast(1, 1), lambda: em_x_bn(1, 1), lambda: em_x_T(1, 1)),
        2: (lambda: em_x_cast(1, 2), lambda: em_x_bn(1, 2), lambda: em_x_T(1, 2)),
        3: (lambda: em_x_cast(1, 3), lambda: em_x_bn(1, 3), lambda: em_x_T(1, 3),
            lambda: em_rsq(1, 0)),
        4: (lambda: em_x_cast(1, 4), lambda: em_x_bn(1, 4), lambda: em_x_T(1, 4),
            lambda: em_rmu(1, 0), lambda: em_gath(1, 0)),
        5: (lambda: em_x_cast(1, 5), lambda: em_x_bn(1, 5), lambda: em_x_T(1, 5),
            lambda: em_bcast(1, 0)),
        6: (lambda: em_x_cast(1, 6), lambda: em_x_bn(1, 6), lambda: em_x_T(1, 6),
            lambda: em_rsq(1, 1)),
        7: (lambda: em_x_cast(1, 7), lambda: em_x_bn(1, 7), lambda: em_x_T(1, 7),
            lambda: em_rmu(1, 1), lambda: em_gath(1, 1)),
    }
    em_proj_cp(1, 0, WT4_t, 1, tpT, extras_cp1)
    em_bcast(1, 1)
    wraw.release()

    ipT = pT.tile([P, KT, S], DT, tag="ipT")
    extras_icp0 = {
        0: (lambda: em_x_cast(1, 8), lambda: em_x_bn(1, 8), lambda: em_x_T(1, 8)),
        1: (lambda: em_x_cast(1, 9), lambda: em_x_bn(1, 9), lambda: em_x_T(1, 9)),
        2: (lambda: em_x_cast(1, 10), lambda: em_x_bn(1, 10), lambda: em_x_T(1, 10)),
        3: (lambda: em_x_cast(1, 11), lambda: em_x_bn(1, 11), lambda: em_x_T(1, 11),
            lambda: em_rsq(1, 2)),
        4: (lambda: em_x_cast(1, 12), lambda: em_x_bn(1, 12), lambda: em_x_T(1, 12),
            lambda: em_rmu(1, 2), lambda: em_gath(1, 2), lambda: em_bcast(1, 2)),
        5: (lambda: em_x_cast(1, 13), lambda: em_x_bn(1, 13), lambda: em_x_T(1, 13)),
        6: (lambda: em_x_cast(1, 14), lambda: em_x_bn(1, 14), lambda: em_x_T(1, 14),
            lambda: em_rsq(1, 3)),
        7: (lambda: em_x_cast(1, 15), lambda: em_x_bn(1, 15), lambda: em_x_T(1, 15),
            lambda: em_rmu(1, 3), lambda: em_gath(1, 3), lambda: em_bcast(1, 3)),
    }
    em_proj_cp(0, 1, WT4_i, 0, ipT, extras_icp0)
    em_proj_cp(0, 1, WT4_i, 1, ipT, {})

    x16p.release()
    xr.release()
    bcast.release()
    xT.release()
    wT.release()

    tp_pool = tc.alloc_tile_pool(name="tp", bufs=1)
    tp = tp_pool.tile([P, ST, D], DT, tag="tp")
    for et in range(KT):
        nc.sync.dma_start_transpose(tp[:, :, et * P : (et + 1) * P], tpT[:, et, :])

    attn_pool = tc.alloc_tile_pool(name="attn", bufs=1)
    A = attn_pool.tile([P, ST, S], DT, tag="A")
    ip = attn_pool.tile([P, ST, D], DT, tag="ip")
    evq = tc.alloc_tile_pool(name="evq", bufs=2)
    outs = tc.alloc_tile_pool(name="outs", bufs=2)

    # ---- attention: QK + exp + A^T, image_out pipelined one s-tile behind --
    def _emit_io(m, at_full):
        iops = [
            psB.tile([P, CH], F32, tag="io", name=f"io_{m}_{dci}")
            for dci in range(DCH)
        ]
        for tt in range(ST):
            for dc in range(DCH):
                nc.tensor.matmul(
                    iops[dc][:],
                    lhsT=at_full[:, tt, :],
                    rhs=tp[:, tt, dc * CH : (dc + 1) * CH],
                    start=(tt == 0),
                    stop=(tt == ST - 1),
                )
        iosb = outs.tile([P, D], DT, tag="osb", name=f"iosb_{m}")
        for dc in range(DCH):
            nc.vector.tensor_copy(iosb[:, dc * CH : (dc + 1) * CH], iops[dc][:])
        nc.gpsimd.dma_start(io_out[m * P : (m + 1) * P, :], iosb[:])

    pending_io = None
    for m in range(ST):
        qps = [
            psA.tile([P, CH], F32, tag="mm", name=f"qk_{m}_{ci}")
            for ci in range(NCH)
        ]
        for kk in range(KT):
            for ci in range(NCH):
                nc.tensor.matmul(
                    qps[ci][:],
                    lhsT=ipT[:, kk, m * P : (m + 1) * P],
                    rhs=tpT[:, kk, ci * CH : (ci + 1) * CH],
                    start=(kk == 0),
                    stop=(kk == KT - 1),
                )
        rs4 = stats.tile([P, NCH], F32, tag="rs4")
        for ci in range(NCH):
            nc.scalar.activation(
                A[:, m, ci * CH : (ci + 1) * CH],
                qps[ci][:],
                ACTF.Exp,
                bias=0.0,
                scale=scale_t[:],
                accum_out=rs4[:, ci : ci + 1],
            )
        rsum = stats.tile([P, 1], F32, tag="rsum")
        nc.vector.reduce_sum(rsum[:], rs4[:], axis=AXL.X)
        nc.vector.reciprocal(rinv[:, m : m + 1], rsum[:])
        nc.vector.tensor_scalar_mul(A[:, m, :], A[:, m, :], rinv[:, m : m + 1])
        at_full = evq.tile([P, ST, P], DT, tag="at", name=f"at_{m}")
        nc.sync.dma_start_transpose(at_full[:, :, :], A[:, m, :])
        # image natural layout [s, e] built during the QK phase (sync queue)
        if m < KT:
            nc.sync.dma_start_transpose(ip[:, :, m * P : (m + 1) * P], ipT[:, m, :])
        if pending_io is not None:
            _emit_io(*pending_io)
        pending_io = (m, at_full)
    _emit_io(*pending_io)
    pending_io = None

    # ---- text_out: single pass, full-A ----
    for tt in range(ST):
        tops = [
            psB.tile([P, CH], F32, tag="io", name=f"to_{tt}_{dci}")
            for dci in range(DCH)
        ]
        for ss in range(ST):
            for dc in range(DCH):
                nc.tensor.matmul(
                    tops[dc][:],
                    lhsT=A[:, ss, tt * P : (tt + 1) * P],
                    rhs=ip[:, ss, dc * CH : (dc + 1) * CH],
                    start=(ss == 0),
                    stop=(ss == ST - 1),
                )
        tosb = outs.tile([P, D], DT, tag="osb", name=f"tosb_{tt}")
        for dc in range(DCH):
            nc.vector.tensor_copy(tosb[:, dc * CH : (dc + 1) * CH], tops[dc][:])
        nc.gpsimd.dma_start(to_out[tt * P : (tt + 1) * P, :], tosb[:])

    for p in (outs, evq, attn_pool, tp_pool, pT, psB, psA, dram, stats, persist):
        p.release()


_NC_CACHE = {}


def build_nc():
    if "nc" not in _NC_CACHE:
        nc = bacc.Bacc("TRN2", target_bir_lowering=False, debug=False)
        with tile.TileContext(nc) as tc:
            _body(tc)
        nc.compile()
        _NC_CACHE["nc"] = nc
    return _NC_CACHE["nc"]


def _in_maps(image_features, text_features, ln_w, ln_b, W_img, b_img, W_txt, b_txt):
    f32 = lambda a: np.ascontiguousarray(np.asarray(a), dtype=np.float32)
    shared = {
        "ln_w": f32(ln_w),
        "ln_b": f32(ln_b),
        "W_img": f32(W_img),
        "b_img": f32(b_img),
        "W_txt": f32(W_txt),
        "b_txt": f32(b_txt),
    }
    maps = []
    for b in range(NCORES):
        m = dict(shared)
        m["image_features"] = f32(image_features[b])
        m["text_features"] = f32(text_features[b])
        maps.append(m)
    return maps


def run(inputs, trace=False, tmpdir=None):
    nc = build_nc()
    maps = _in_maps(**inputs)
    res = run_bass_kernel_spmd(
        nc, maps, core_ids=list(range(NCORES)), trace=trace, tmpdir=tmpdir
    )
    io = np.stack([res.results[b]["image_out"] for b in range(NCORES)])
    to = np.stack([res.results[b]["text_out"] for b in range(NCORES)])
    return (io, to), res


def kernel(**inputs):
    out, _ = run(inputs, trace=False)
    return out
